# revision 16
# baseline (speedup 1.0000x reference)
"""Fused vocab-parallel ArcMarginProduct + CrossEntropy loss on 8 TRN2 NeuronCores.

Strategy: shard the class dimension C across 8 cores (tensor/vocab parallel).
Each core normalizes its weight shard, computes bf16 logits fn_hat @ wn_hat.T
for its 12544 classes, exponentiates with a fixed max bound (logits <= S=30)
while accumulating row sums on the scalar engine, gathers its resident target
rows to compute the exact target cosine in f32, and a single 8KB AllReduce
combines [row-sums || target-cos].  The ArcFace margin (phi) and the final
log-softmax correction are computed redundantly on every core.
"""

import math

import numpy as np

import concourse.bass as bass
import concourse.mybir as mybir
import concourse.tile as tile
from concourse.bass_utils import run_bass_kernel_spmd

# Problem constants (hardcoded per harness contract)
B, D, C = 1024, 512, 100000
S = 30.0
M = 0.3
COS_M = math.cos(M)
SIN_M = math.sin(M)
TH = math.cos(math.pi - M)
MM = math.sin(math.pi - M) * M

NCORES = 8
CPAD = 100352            # C padded to 8 * 28 * 448
CS = CPAD // NCORES      # 12544 classes per core
NPAD = CPAD - C          # 352 zero-padded classes (on core 7)
CHUNK = 448              # c-chunk width (<=512, PSUM bank friendly)
NCH = CS // CHUNK        # 28 chunks per core
NB = B // 128            # 8 batch tiles
NK = D // 128            # 4 contraction tiles
GW = 2                   # chunks per psum group
NG = NCH // GW           # 14 groups
MAXL = 30.0              # fixed logit max bound (cos <= 1, S = 30)
EPS_N = 1e-24            # epsilon inside ln() for zero-padded rows
PAD_CORR = NPAD * math.exp(-MAXL)   # padded columns' exp contribution

F32 = mybir.dt.float32
F32R = mybir.dt.float32r
BF16 = mybir.dt.bfloat16
I32 = mybir.dt.int32
AX = mybir.AxisListType.X
OP = mybir.AluOpType
AF = mybir.ActivationFunctionType


def _patch_tail_drain():
    """This walrus build rejects >2 sync waits on one CTRL instruction
    ("Too many sync wait commands").  TileContext's tail drain accumulates one
    wait per logical proc; split them across multiple drain instructions."""
    import bass_rust
    from concourse.tile import ScopedClock, TileContext

    if getattr(TileContext, "_tail_drain_split", False):
        return

    def _drain_and_barrier(self, tick_clock, wait_clock):
        nc = self.nc
        drain_inst = nc.sync.drain()
        wait_clock.add_sem_waits(
            drain_inst.ins, ScopedClock({None: tick_clock.global_clock})
        )
        si = drain_inst.ins.sync_info
        if si is not None and len(si.on_wait) > 1:
            waits = list(si.on_wait)
            si.on_wait = waits[:1]
            for w in waits[1:]:
                extra = nc.sync.drain()
                extra.ins.sync_info = bass_rust.SyncInfo(
                    on_wait=[w], on_update=[])
        nc.all_engine_barrier()
        assert self.sems is not None
        popped = nc._tile_sem_poison_stack.pop()
        assert popped is self._sem_poison
        nc.clear_and_free_semaphores(list(self.sems.allocated().values()))
        nc.all_engine_barrier()

    TileContext._drain_and_barrier = _drain_and_barrier
    TileContext._tail_drain_split = True


_patch_tail_drain()


def _patch_ldw_opt():
    """Re-enable walrus LDWEIGHTS dedup so consecutive matmuls sharing a
    stationary operand load it once."""
    import concourse.bass_utils as bu
    if getattr(bu, "_ldw_patched", False):
        return
    orig = bu.run_command
    def run_command(cmd, *a, **kw):
        cmd = ["--enable-ldw-opt=true" if c == "--enable-ldw-opt=false" else c
               for c in cmd]
        return orig(cmd, *a, **kw)
    bu.run_command = run_command
    bu._ldw_patched = True




def _dedup_ldweights(nc):
    """Tile emits one Ldweights per matmul.  Consecutive loads of the same
    stationary AP (only Matmult/NoOp between) are redundant — the PE keeps
    the stationary operand until the next load.  Drop them; preserve any
    sem waits/updates on a NoOp."""
    import bass_rust

    dropped = 0
    for f in nc.m.functions:
        for blk in f.blocks:
            out = []
            prev_sig = None
            changed = False
            for inst in blk.instructions:
                tname = type(inst).__name__
                if tname == "InstLdweights":
                    sig = str(inst.ins[0])
                    if sig == prev_sig:
                        si = getattr(inst, "sync_info", None)
                        has_sync = si is not None and (
                            (si.on_wait and len(si.on_wait)) or
                            (si.on_update and len(si.on_update)))
                        if has_sync:
                            nop = bass_rust.InstNoOp(
                                name=f"I-ldwnop{dropped}", engine=inst.engine)
                            nop.sync_info = si
                            out.append(nop)
                        dropped += 1
                        changed = True
                        continue
                    prev_sig = sig
                elif tname == "InstMatmult":
                    pass  # keeps stationary operand
                elif tname == "InstNoOp":
                    pass
                elif str(getattr(inst, "engine", "")) == "EngineType.PE":
                    prev_sig = None
                out.append(inst)
            if changed:
                blk.instructions = out
    return dropped


def _split_excess_waits(nc, max_waits=1):
    """Walrus here encodes at most one sync-wait on several instruction
    structs.  Move excess waits onto preceding same-engine NoOps (the engine
    stalls at the NoOp instead; semantics identical for sem-ge waits)."""
    import bass_rust

    n_split = 0
    for f in nc.m.functions:
        for blk in f.blocks:
            out = []
            changed = False
            for inst in blk.instructions:
                si = getattr(inst, "sync_info", None)
                waits = list(si.on_wait) if si is not None and si.on_wait else []
                if len(waits) > max_waits:
                    for w in waits[:-max_waits]:
                        nop = bass_rust.InstNoOp(
                            name=f"I-wsp{n_split}", engine=inst.engine)
                        nop.sync_info = bass_rust.SyncInfo(
                            on_wait=[w], on_update=[])
                        out.append(nop)
                        n_split += 1
                    si.on_wait = waits[-max_waits:]
                    changed = True
                out.append(inst)
            if changed:
                blk.instructions = out
    return n_split


def build_graph(split_waits=True):
    nc = bass.Bass()

    feat = nc.declare_dram_parameter("feat", [B, D], F32, isOutput=False)
    featT = nc.declare_dram_parameter("featT", [D, B], F32, isOutput=False)
    wt = nc.declare_dram_parameter("wt", [D, CS], F32, isOutput=False)
    wsh = nc.declare_dram_parameter("wsh", [CS, D], F32, isOutput=False)
    tloc = nc.declare_dram_parameter("tloc", [B], I32, isOutput=False)
    out_ext = nc.declare_dram_parameter("out", [1, 1], F32, isOutput=True)

    with tile.TileContext(nc) as tc:
        with (
            tc.tile_pool(name="persist", bufs=1) as pp,
            tc.tile_pool(name="wstage", bufs=4) as wsp,
            tc.tile_pool(name="sqpool", bufs=4) as sqp,
            tc.tile_pool(name="ninvp", bufs=4) as nip,
            tc.tile_pool(name="gathp", bufs=3) as gp,
            tc.tile_pool(name="trashp", bufs=2) as trp,
            tc.tile_pool(name="expop", bufs=3) as xp,
            tc.tile_pool(name="ft32p", bufs=2) as ftp,
            tc.tile_pool(name="smallp", bufs=2) as sp,
            tc.tile_pool(name="psum_mm", bufs=3, space="PSUM") as pmm,
            tc.tile_pool(name="psum_n", bufs=2, space="PSUM") as pn,
            tc.tile_pool(name="dramp", bufs=1, space="DRAM") as dp,
        ):
            # ---- constants ----
            ones_k = pp.tile([128, 1], F32, name="ones_k")
            nc.vector.memset(ones_k[:], 1.0)
            ones_m = pp.tile([128, 128], BF16, name="ones_m")
            nc.vector.memset(ones_m[:], 1.0)
            eps_b = pp.tile([128, 1], F32, name="eps_b")
            nc.vector.memset(eps_b[:], EPS_N)
            lnS_b = pp.tile([128, 1], F32, name="lnS_b")
            nc.vector.memset(lnS_b[:], math.log(S))
            negmax_b = pp.tile([128, 1], F32, name="negmax_b")
            nc.vector.memset(negmax_b[:], -MAXL)

            # ---- A. features: load, sum-of-squares, s_scale = S * inv_norm ----
            feat_sb = []
            for j in range(NB):
                fsb = pp.tile([128, D], F32, name=f"feat_sb{j}")
                nc.sync.dma_start(out=fsb[:], in_=feat[j * 128:(j + 1) * 128, :])
                feat_sb.append(fsb)
            fss = pp.tile([128, NB], F32, name="fss")
            for j in range(NB):
                ftrash = trp.tile([128, D], BF16, name="ftrash", tag="ttrash")
                nc.vector.scalar_tensor_tensor(
                    out=ftrash[:], in0=feat_sb[j][:], scalar=1.0,
                    in1=feat_sb[j][:], op0=OP.mult, op1=OP.mult,
                    accum_out=fss[:, j:j + 1],
                )
            ln_fss = pp.tile([128, NB], F32, name="ln_fss")
            nc.scalar.activation(ln_fss[:], fss[:], AF.Ln, bias=eps_b[:])
            s_scale = pp.tile([128, NB], F32, name="s_scale")
            # S * fss^-0.5 = exp(-0.5*ln(fss) + ln(S))
            nc.scalar.activation(s_scale[:], ln_fss[:], AF.Exp,
                                 bias=lnS_b[:], scale=-0.5)
            inv_f = pp.tile([128, NB], F32, name="inv_f")
            nc.scalar.activation(inv_f[:], ln_fss[:], AF.Exp, bias=0.0, scale=-0.5)

            # ---- B. featT -> bf16 lhsT tiles ----
            fT = []
            for k in range(NK):
                f32t = ftp.tile([128, B], F32, name="f32t")
                nc.sync.dma_start(out=f32t[:], in_=featT[k * 128:(k + 1) * 128, :])
                fTk = pp.tile([128, B], BF16, name=f"fT{k}")
                nc.vector.tensor_copy(out=fTk[:], in_=f32t[:])
                fT.append(fTk)

            # ---- E1. target gather + f32 dot products ----
            tl_i = pp.tile([128, NB], I32, name="tl_i")
            nc.sync.dma_start(
                out=tl_i[:],
                in_=tloc.rearrange("(j p) -> p j", p=128),
            )
            tl_f = pp.tile([128, NB], F32, name="tl_f")
            nc.vector.tensor_copy(out=tl_f[:], in_=tl_i[:])
            mask0 = sp.tile([128, NB], F32, name="mask0")
            nc.vector.tensor_scalar(out=mask0[:], in0=tl_f[:], scalar1=0.0,
                                    scalar2=None, op0=OP.is_ge)
            mask1 = sp.tile([128, NB], F32, name="mask1")
            nc.vector.tensor_scalar(out=mask1[:], in0=tl_f[:], scalar1=float(CS - 1),
                                    scalar2=None, op0=OP.is_le)
            tmask = pp.tile([128, NB], F32, name="tmask")
            nc.vector.tensor_tensor(out=tmask[:], in0=mask0[:], in1=mask1[:],
                                    op=OP.mult)
            idx0 = sp.tile([128, NB], I32, name="idx0")
            nc.vector.tensor_scalar(out=idx0[:], in0=tl_i[:], scalar1=0,
                                    scalar2=None, op0=OP.max)
            idx_safe = pp.tile([128, NB], I32, name="idx_safe")
            nc.vector.tensor_scalar(out=idx_safe[:], in0=idx0[:], scalar1=CS - 1,
                                    scalar2=None, op0=OP.min)

            gss = pp.tile([128, NB], F32, name="gss")
            gdot = pp.tile([128, NB], F32, name="gdot")
            for j in range(NB):
                gtile = gp.tile([128, D], F32, name="gtile")
                nc.gpsimd.indirect_dma_start(
                    out=gtile[:],
                    out_offset=None,
                    in_=wsh[:],
                    in_offset=bass.IndirectOffsetOnAxis(
                        ap=idx_safe[:, j:j + 1], axis=0),
                )
                gtrash = trp.tile([128, D], BF16, name="gtrash", tag="ttrash")
                nc.vector.scalar_tensor_tensor(
                    out=gtrash[:], in0=gtile[:], scalar=1.0,
                    in1=gtile[:], op0=OP.mult, op1=OP.mult,
                    accum_out=gss[:, j:j + 1],
                )
                gtrash2 = trp.tile([128, D], BF16, name="gtrash2", tag="ttrash")
                nc.vector.scalar_tensor_tensor(
                    out=gtrash2[:], in0=gtile[:], scalar=1.0,
                    in1=feat_sb[j][:], op0=OP.mult, op1=OP.mult,
                    accum_out=gdot[:, j:j + 1],
                )
            ln_gss = sp.tile([128, NB], F32, name="ln_gss")
            nc.scalar.activation(ln_gss[:], gss[:], AF.Ln, bias=eps_b[:])
            inv_g = sp.tile([128, NB], F32, name="inv_g")
            nc.scalar.activation(inv_g[:], ln_gss[:], AF.Exp, bias=0.0, scale=-0.5)
            t0 = sp.tile([128, NB], F32, name="t0")
            nc.vector.tensor_tensor(out=t0[:], in0=gdot[:], in1=inv_g[:], op=OP.mult)
            t1 = sp.tile([128, NB], F32, name="t1")
            nc.vector.tensor_tensor(out=t1[:], in0=t0[:], in1=inv_f[:], op=OP.mult)
            t_m = pp.tile([128, NB], F32, name="t_m")
            nc.vector.tensor_tensor(out=t_m[:], in0=t1[:], in1=tmask[:], op=OP.mult)
            tin = dp.tile([128, NB], F32, name="tin")
            tout = dp.tile([128, NB], F32, name="tout", addr_space="Shared")
            nc.sync.dma_start(out=tin[:], in_=t_m[:])
            nc.gpsimd.collective_compute(
                "AllReduce", OP.add,
                replica_groups=[list(range(NCORES))],
                ins=[tin[:]], outs=[tout[:]],
            )
            t_all = pp.tile([128, NB], F32, name="t_all")
            nc.sync.dma_start(out=t_all[:], in_=tout[:])

            # ---- C. weight shard pipeline (bf16): norms + normalize ----
            wTn = []
            for k in range(NK):
                wTnk = pp.tile([128, CS], BF16, name=f"wTn{k}")
                wTn.append(wTnk)
            for pr in range(NCH // 2):
                c0 = pr * 2 * CHUNK
                wsp2, wsb2, sq2 = [], [], []
                for k in range(NK):
                    wsk = wsp.tile([128, 2 * CHUNK], F32, name="wsk",
                                   tag="wstage")
                    nc.sync.dma_start(
                        out=wsk[:],
                        in_=wt[k * 128:(k + 1) * 128, c0:c0 + 2 * CHUNK])
                    wsp2.append(wsk)
                for k in range(NK):
                    wsbk = sqp.tile([128, 2 * CHUNK], BF16, name="wsbk",
                                    tag="wsb")
                    nc.vector.tensor_copy(out=wsbk[:], in_=wsp2[k][:])
                    wsb2.append(wsbk)
                for k in range(NK):
                    sqk = sqp.tile([128, 2 * CHUNK], BF16, name="sqk", tag="sq")
                    nc.vector.tensor_tensor(out=sqk[:], in0=wsb2[k][:],
                                            in1=wsb2[k][:], op=OP.mult)
                    sq2.append(sqk)
                for half in range(2):
                    cs0 = c0 + half * CHUNK
                    hs = slice(half * CHUNK, (half + 1) * CHUNK)
                    # norm matmul with all-ones lhsT broadcasts the column
                    # sums to every output partition
                    npsum = pn.tile([128, CHUNK], F32, name="npsum",
                                    tag="npsum")
                    for k in range(NK):
                        nc.tensor.matmul(
                            out=npsum[:],
                            lhsT=ones_m[:],
                            rhs=sq2[k][:, hs],
                            start=(k == 0), stop=(k == NK - 1),
                        )
                    lnn = nip.tile([128, CHUNK], F32, name="lnn", tag="lnn")
                    nc.scalar.activation(lnn[:], npsum[:], AF.Ln, bias=eps_b[:])
                    ninvb = nip.tile([128, CHUNK], BF16, name="ninvb",
                                     tag="ninvb")
                    nc.scalar.activation(ninvb[:], lnn[:], AF.Exp,
                                         bias=0.0, scale=-0.5)
                    for k in range(NK):
                        nc.vector.tensor_tensor(
                            out=wTn[k][:, cs0:cs0 + CHUNK],
                            in0=wsb2[k][:, hs], in1=ninvb[:], op=OP.mult)

            # ---- D. main matmuls + fused exp/row-sum epilogue ----
            r_parts = pp.tile([128, NB * NG], F32, name="r_parts")
            r_sum = pp.tile([128, NB], F32, name="r_sum")
            agg_r = pp.tile([128, NB], F32, name="agg_r")
            rins, routs = [], []
            for j in range(NB):
                rins.append(dp.tile([128, 1], F32, name=f"rin{j}"))
                routs.append(dp.tile([128, 1], F32, name=f"rout{j}",
                                     addr_space="Shared"))
            for j in range(NB):
                for q in range(NG // 2):
                    pa = pmm.tile([128, GW, CHUNK], F32, name="pa",
                                  tag="mm_ps", padded_shape=[128, GW, 512])
                    pb = pmm.tile([128, GW, CHUNK], F32, name="pb",
                                  tag="mm_ps", padded_shape=[128, GW, 512])
                    for k in range(NK):
                        lhs = fT[k][:, j * 128:(j + 1) * 128]
                        for t, pt in enumerate((pa, pb)):
                            for cc2 in range(GW):
                                c0 = ((q * 2 + t) * GW + cc2) * CHUNK
                                nc.tensor.matmul(
                                    out=pt[:, cc2, :],
                                    lhsT=lhs,
                                    rhs=wTn[k][:, c0:c0 + CHUNK],
                                    start=(k == 0), stop=(k == NK - 1),
                                )
                    for t, pt in enumerate((pa, pb)):
                        g = q * 2 + t
                        expo = xp.tile([128, GW, CHUNK], BF16, name="expo",
                                       tag="expo")
                        nc.scalar.activation(
                            expo[:], pt[:], AF.Exp,
                            bias=negmax_b[:], scale=s_scale[:, j:j + 1],
                            accum_out=r_parts[:, j * NG + g:j * NG + g + 1],
                        )
                nc.vector.reduce_sum(
                    out=r_sum[:, j:j + 1],
                    in_=r_parts[:, j * NG:(j + 1) * NG], axis=AX)
                nc.sync.dma_start(out=rins[j][:], in_=r_sum[:, j:j + 1])
                nc.gpsimd.collective_compute(
                    "AllReduce", OP.add,
                    replica_groups=[list(range(NCORES))],
                    ins=[rins[j][:]], outs=[routs[j][:]],
                )
                nc.sync.dma_start(out=agg_r[:, j:j + 1], in_=routs[j][:])

            # ---- G. phi + corrected log-softmax + mean (redundant on all cores) --
            t2 = sp.tile([128, NB], F32, name="t2")
            nc.vector.tensor_tensor(out=t2[:], in0=t_all[:], in1=t_all[:], op=OP.mult)
            t2c = sp.tile([128, NB], F32, name="t2c")
            nc.vector.tensor_scalar(out=t2c[:], in0=t2[:], scalar1=1.0,
                                    scalar2=None, op0=OP.min)
            ln_u = sp.tile([128, NB], F32, name="ln_u")
            nc.scalar.activation(ln_u[:], t2c[:], AF.Ln, bias=1.0, scale=-1.0)
            sine = sp.tile([128, NB], F32, name="sine")
            nc.scalar.activation(sine[:], ln_u[:], AF.Exp, bias=0.0, scale=0.5)
            pa_ = sp.tile([128, NB], F32, name="pa_")
            nc.vector.tensor_scalar(out=pa_[:], in0=t_all[:], scalar1=COS_M,
                                    scalar2=None, op0=OP.mult)
            pb_ = sp.tile([128, NB], F32, name="pb_")
            nc.vector.tensor_scalar(out=pb_[:], in0=sine[:], scalar1=SIN_M,
                                    scalar2=None, op0=OP.mult)
            phi_m = sp.tile([128, NB], F32, name="phi_m")
            nc.vector.tensor_tensor(out=phi_m[:], in0=pa_[:], in1=pb_[:],
                                    op=OP.subtract)
            phi_alt = sp.tile([128, NB], F32, name="phi_alt")
            nc.vector.tensor_scalar(out=phi_alt[:], in0=t_all[:], scalar1=MM,
                                    scalar2=None, op0=OP.subtract)
            thmask = sp.tile([128, NB], I32, name="thmask")
            nc.vector.tensor_scalar(out=thmask[:], in0=t_all[:], scalar1=TH,
                                    scalar2=None, op0=OP.is_gt)
            phi = sp.tile([128, NB], F32, name="phi")
            nc.vector.select(out=phi[:], mask=thmask[:], on_true=phi_m[:],
                             on_false=phi_alt[:])
            e_t = sp.tile([128, NB], F32, name="e_t")
            nc.scalar.activation(e_t[:], t_all[:], AF.Exp, bias=negmax_b[:], scale=S)
            e_phi = sp.tile([128, NB], F32, name="e_phi")
            nc.scalar.activation(e_phi[:], phi[:], AF.Exp, bias=negmax_b[:], scale=S)
            rc = sp.tile([128, NB], F32, name="rc")
            nc.vector.tensor_tensor(out=rc[:], in0=agg_r[:], in1=e_t[:],
                                    op=OP.subtract)
            rc2 = sp.tile([128, NB], F32, name="rc2")
            nc.vector.tensor_tensor(out=rc2[:], in0=rc[:], in1=e_phi[:], op=OP.add)
            rc3 = sp.tile([128, NB], F32, name="rc3")
            nc.vector.tensor_scalar(out=rc3[:], in0=rc2[:], scalar1=PAD_CORR,
                                    scalar2=None, op0=OP.subtract)
            ln_r = sp.tile([128, NB], F32, name="ln_r")
            nc.scalar.activation(ln_r[:], rc3[:], AF.Ln)
            lp = sp.tile([128, NB], F32, name="lp")
            nc.vector.tensor_scalar(out=lp[:], in0=phi[:], scalar1=S,
                                    scalar2=-MAXL, op0=OP.mult, op1=OP.add)
            ll = sp.tile([128, NB], F32, name="ll")
            nc.vector.tensor_tensor(out=ll[:], in0=ln_r[:], in1=lp[:],
                                    op=OP.subtract)
            lsum = sp.tile([128, 1], F32, name="lsum")
            nc.vector.reduce_sum(out=lsum[:], in_=ll[:], axis=AX)
            loss_ps = pn.tile([1, 1], F32, name="loss_ps", tag="npsum")
            nc.tensor.matmul(out=loss_ps[:], lhsT=lsum[:], rhs=ones_k[:],
                             start=True, stop=True)
            loss_sb = sp.tile([1, 1], F32, name="loss_sb")
            nc.scalar.activation(loss_sb[:], loss_ps[:], AF.Copy, scale=1.0 / B)
            nc.sync.dma_start(out=out_ext[:], in_=loss_sb[:])

    if split_waits:
        _dedup_ldweights(nc)
        _split_excess_waits(nc)
    return nc


_CACHE = {}


def make_in_maps(features, weight, targets):
    feats = np.ascontiguousarray(np.asarray(features, dtype=np.float32))
    W = np.asarray(weight, dtype=np.float32)
    tg = np.asarray(targets).astype(np.int64)

    featT = np.ascontiguousarray(feats.T)
    Wpad = np.zeros((CPAD, D), dtype=np.float32)
    Wpad[:C] = W

    in_maps = []
    for r in range(NCORES):
        wsh = np.ascontiguousarray(Wpad[r * CS:(r + 1) * CS])
        wtr = np.ascontiguousarray(wsh.T)
        tl = (tg - r * CS).astype(np.int32)
        in_maps.append({
            "feat": feats,
            "featT": featT,
            "wt": wtr,
            "wsh": wsh,
            "tloc": tl,
        })
    return in_maps


def kernel(features, weight, targets):
    in_maps = make_in_maps(features, weight, targets)
    if "nc" not in _CACHE:
        _CACHE["nc"] = build_graph()
    nc = _CACHE["nc"]
    res = run_bass_kernel_spmd(nc, in_maps, core_ids=list(range(NCORES)))
    return np.float32(res.results[0]["out"][0, 0])


# revision 17
# speedup vs baseline: 1.0555x; 1.0555x over previous
"""Fused vocab-parallel ArcMarginProduct + CrossEntropy loss on 8 TRN2 NeuronCores.

Strategy: shard the class dimension C across 8 cores (tensor/vocab parallel).
Each core normalizes its weight shard, computes bf16 logits fn_hat @ wn_hat.T
for its 12544 classes, exponentiates with a fixed max bound (logits <= S=30)
while accumulating row sums on the scalar engine, gathers its resident target
rows to compute the exact target cosine in f32, and a single 8KB AllReduce
combines [row-sums || target-cos].  The ArcFace margin (phi) and the final
log-softmax correction are computed redundantly on every core.
"""

import math

import numpy as np

import concourse.bass as bass
import concourse.mybir as mybir
import concourse.tile as tile
from concourse.bass_utils import run_bass_kernel_spmd

# Problem constants (hardcoded per harness contract)
B, D, C = 1024, 512, 100000
S = 30.0
M = 0.3
COS_M = math.cos(M)
SIN_M = math.sin(M)
TH = math.cos(math.pi - M)
MM = math.sin(math.pi - M) * M

NCORES = 8
CPAD = 100352            # C padded to 8 * 28 * 448
CS = CPAD // NCORES      # 12544 classes per core
NPAD = CPAD - C          # 352 zero-padded classes (on core 7)
CHUNK = 448              # c-chunk width (<=512, PSUM bank friendly)
NCH = CS // CHUNK        # 28 chunks per core
NB = B // 128            # 8 batch tiles
NK = D // 128            # 4 contraction tiles
GW = 2                   # chunks per psum group
NG = NCH // GW           # 14 groups
MAXL = 30.0              # fixed logit max bound (cos <= 1, S = 30)
EPS_N = 1e-24            # epsilon inside ln() for zero-padded rows
PAD_CORR = NPAD * math.exp(-MAXL)   # padded columns' exp contribution

F32 = mybir.dt.float32
F32R = mybir.dt.float32r
BF16 = mybir.dt.bfloat16
FP8 = mybir.dt.float8e4
I32 = mybir.dt.int32
AX = mybir.AxisListType.X
OP = mybir.AluOpType
AF = mybir.ActivationFunctionType


def _patch_tail_drain():
    """This walrus build rejects >2 sync waits on one CTRL instruction
    ("Too many sync wait commands").  TileContext's tail drain accumulates one
    wait per logical proc; split them across multiple drain instructions."""
    import bass_rust
    from concourse.tile import ScopedClock, TileContext

    if getattr(TileContext, "_tail_drain_split", False):
        return

    def _drain_and_barrier(self, tick_clock, wait_clock):
        nc = self.nc
        drain_inst = nc.sync.drain()
        wait_clock.add_sem_waits(
            drain_inst.ins, ScopedClock({None: tick_clock.global_clock})
        )
        si = drain_inst.ins.sync_info
        if si is not None and len(si.on_wait) > 1:
            waits = list(si.on_wait)
            si.on_wait = waits[:1]
            for w in waits[1:]:
                extra = nc.sync.drain()
                extra.ins.sync_info = bass_rust.SyncInfo(
                    on_wait=[w], on_update=[])
        nc.all_engine_barrier()
        assert self.sems is not None
        popped = nc._tile_sem_poison_stack.pop()
        assert popped is self._sem_poison
        nc.clear_and_free_semaphores(list(self.sems.allocated().values()))
        nc.all_engine_barrier()

    TileContext._drain_and_barrier = _drain_and_barrier
    TileContext._tail_drain_split = True


_patch_tail_drain()


def _patch_ldw_opt():
    """Re-enable walrus LDWEIGHTS dedup so consecutive matmuls sharing a
    stationary operand load it once."""
    import concourse.bass_utils as bu
    if getattr(bu, "_ldw_patched", False):
        return
    orig = bu.run_command
    def run_command(cmd, *a, **kw):
        cmd = ["--enable-ldw-opt=true" if c == "--enable-ldw-opt=false" else c
               for c in cmd]
        return orig(cmd, *a, **kw)
    bu.run_command = run_command
    bu._ldw_patched = True




def _dedup_ldweights(nc):
    """Tile emits one Ldweights per matmul.  Consecutive loads of the same
    stationary AP (only Matmult/NoOp between) are redundant — the PE keeps
    the stationary operand until the next load.  Drop them; preserve any
    sem waits/updates on a NoOp."""
    import bass_rust

    dropped = 0
    for f in nc.m.functions:
        for blk in f.blocks:
            out = []
            prev_sig = None
            changed = False
            for inst in blk.instructions:
                tname = type(inst).__name__
                if tname == "InstLdweights":
                    sig = str(inst.ins[0])
                    if sig == prev_sig:
                        si = getattr(inst, "sync_info", None)
                        has_sync = si is not None and (
                            (si.on_wait and len(si.on_wait)) or
                            (si.on_update and len(si.on_update)))
                        if has_sync:
                            nop = bass_rust.InstNoOp(
                                name=f"I-ldwnop{dropped}", engine=inst.engine)
                            nop.sync_info = si
                            out.append(nop)
                        dropped += 1
                        changed = True
                        continue
                    prev_sig = sig
                elif tname == "InstMatmult":
                    pass  # keeps stationary operand
                elif tname == "InstNoOp":
                    pass
                elif str(getattr(inst, "engine", "")) == "EngineType.PE":
                    prev_sig = None
                out.append(inst)
            if changed:
                blk.instructions = out
    return dropped


def _split_excess_waits(nc, max_waits=1):
    """Walrus here encodes at most one sync-wait on several instruction
    structs.  Move excess waits onto preceding same-engine NoOps (the engine
    stalls at the NoOp instead; semantics identical for sem-ge waits)."""
    import bass_rust

    n_split = 0
    for f in nc.m.functions:
        for blk in f.blocks:
            out = []
            changed = False
            for inst in blk.instructions:
                si = getattr(inst, "sync_info", None)
                waits = list(si.on_wait) if si is not None and si.on_wait else []
                if len(waits) > max_waits:
                    for w in waits[:-max_waits]:
                        nop = bass_rust.InstNoOp(
                            name=f"I-wsp{n_split}", engine=inst.engine)
                        nop.sync_info = bass_rust.SyncInfo(
                            on_wait=[w], on_update=[])
                        out.append(nop)
                        n_split += 1
                    si.on_wait = waits[-max_waits:]
                    changed = True
                out.append(inst)
            if changed:
                blk.instructions = out
    return n_split


def build_graph(split_waits=True):
    nc = bass.Bass()

    feat = nc.declare_dram_parameter("feat", [B, D], F32, isOutput=False)
    featT = nc.declare_dram_parameter("featT", [D, B], F32, isOutput=False)
    wt = nc.declare_dram_parameter("wt", [D, CS], F32, isOutput=False)
    wsh = nc.declare_dram_parameter("wsh", [CS, D], F32, isOutput=False)
    tloc = nc.declare_dram_parameter("tloc", [B], I32, isOutput=False)
    out_ext = nc.declare_dram_parameter("out", [1, 1], F32, isOutput=True)

    with tile.TileContext(nc) as tc:
        with (
            tc.tile_pool(name="persist", bufs=1) as pp,
            tc.tile_pool(name="wstage", bufs=6) as wsp,
            tc.tile_pool(name="sqpool", bufs=6) as sqp,
            tc.tile_pool(name="ninvp", bufs=4) as nip,
            tc.tile_pool(name="gathp", bufs=3) as gp,
            tc.tile_pool(name="trashp", bufs=2) as trp,
            tc.tile_pool(name="expop", bufs=3) as xp,
            tc.tile_pool(name="ft32p", bufs=2) as ftp,
            tc.tile_pool(name="smallp", bufs=2) as sp,
            tc.tile_pool(name="psum_mm", bufs=3, space="PSUM") as pmm,
            tc.tile_pool(name="psum_n", bufs=2, space="PSUM") as pn,
            tc.tile_pool(name="dramp", bufs=1, space="DRAM") as dp,
        ):
            # ---- constants ----
            ones_k = pp.tile([128, 1], F32, name="ones_k")
            nc.vector.memset(ones_k[:], 1.0)
            ones_m = pp.tile([128, 128], BF16, name="ones_m")
            nc.vector.memset(ones_m[:], 1.0)
            eps_b = pp.tile([128, 1], F32, name="eps_b")
            nc.vector.memset(eps_b[:], EPS_N)
            lnS_b = pp.tile([128, 1], F32, name="lnS_b")
            nc.vector.memset(lnS_b[:], math.log(S))
            negmax_b = pp.tile([128, 1], F32, name="negmax_b")
            nc.vector.memset(negmax_b[:], -MAXL)

            # ---- A. features: load, sum-of-squares, s_scale = S * inv_norm ----
            feat_sb = []
            for j in range(NB):
                fsb = pp.tile([128, D], F32, name=f"feat_sb{j}")
                nc.sync.dma_start(out=fsb[:], in_=feat[j * 128:(j + 1) * 128, :])
                feat_sb.append(fsb)
            fss = pp.tile([128, NB], F32, name="fss")
            for j in range(NB):
                ftrash = trp.tile([128, D], BF16, name="ftrash", tag="ttrash")
                nc.vector.scalar_tensor_tensor(
                    out=ftrash[:], in0=feat_sb[j][:], scalar=1.0,
                    in1=feat_sb[j][:], op0=OP.mult, op1=OP.mult,
                    accum_out=fss[:, j:j + 1],
                )
            ln_fss = pp.tile([128, NB], F32, name="ln_fss")
            nc.scalar.activation(ln_fss[:], fss[:], AF.Ln, bias=eps_b[:])
            s_scale = pp.tile([128, NB], F32, name="s_scale")
            # S * fss^-0.5 = exp(-0.5*ln(fss) + ln(S))
            nc.scalar.activation(s_scale[:], ln_fss[:], AF.Exp,
                                 bias=lnS_b[:], scale=-0.5)
            inv_f = pp.tile([128, NB], F32, name="inv_f")
            nc.scalar.activation(inv_f[:], ln_fss[:], AF.Exp, bias=0.0, scale=-0.5)

            # ---- B. featT -> fp8 DoubleRow lhsT pair tiles ----
            fT8 = []
            for P in range(NK // 2):
                t8 = pp.tile([128, 2, B], FP8, name=f"fT8{P}")
                fT8.append(t8)
            for k in range(NK):
                f32t = ftp.tile([128, B], F32, name="f32t")
                nc.sync.dma_start(out=f32t[:], in_=featT[k * 128:(k + 1) * 128, :])
                nc.vector.tensor_copy(out=fT8[k // 2][:, k % 2, :], in_=f32t[:])

            # ---- E1. target gather + f32 dot products ----
            tl_i = pp.tile([128, NB], I32, name="tl_i")
            nc.sync.dma_start(
                out=tl_i[:],
                in_=tloc.rearrange("(j p) -> p j", p=128),
            )
            tl_f = pp.tile([128, NB], F32, name="tl_f")
            nc.vector.tensor_copy(out=tl_f[:], in_=tl_i[:])
            mask0 = sp.tile([128, NB], F32, name="mask0")
            nc.vector.tensor_scalar(out=mask0[:], in0=tl_f[:], scalar1=0.0,
                                    scalar2=None, op0=OP.is_ge)
            mask1 = sp.tile([128, NB], F32, name="mask1")
            nc.vector.tensor_scalar(out=mask1[:], in0=tl_f[:], scalar1=float(CS - 1),
                                    scalar2=None, op0=OP.is_le)
            tmask = pp.tile([128, NB], F32, name="tmask")
            nc.vector.tensor_tensor(out=tmask[:], in0=mask0[:], in1=mask1[:],
                                    op=OP.mult)
            idx0 = sp.tile([128, NB], I32, name="idx0")
            nc.vector.tensor_scalar(out=idx0[:], in0=tl_i[:], scalar1=0,
                                    scalar2=None, op0=OP.max)
            idx_safe = pp.tile([128, NB], I32, name="idx_safe")
            nc.vector.tensor_scalar(out=idx_safe[:], in0=idx0[:], scalar1=CS - 1,
                                    scalar2=None, op0=OP.min)

            gss = pp.tile([128, NB], F32, name="gss")
            gdot = pp.tile([128, NB], F32, name="gdot")
            for j in range(NB):
                gtile = gp.tile([128, D], F32, name="gtile")
                nc.gpsimd.indirect_dma_start(
                    out=gtile[:],
                    out_offset=None,
                    in_=wsh[:],
                    in_offset=bass.IndirectOffsetOnAxis(
                        ap=idx_safe[:, j:j + 1], axis=0),
                )
                gtrash = trp.tile([128, D], BF16, name="gtrash", tag="ttrash")
                nc.vector.scalar_tensor_tensor(
                    out=gtrash[:], in0=gtile[:], scalar=1.0,
                    in1=gtile[:], op0=OP.mult, op1=OP.mult,
                    accum_out=gss[:, j:j + 1],
                )
                gtrash2 = trp.tile([128, D], BF16, name="gtrash2", tag="ttrash")
                nc.vector.scalar_tensor_tensor(
                    out=gtrash2[:], in0=gtile[:], scalar=1.0,
                    in1=feat_sb[j][:], op0=OP.mult, op1=OP.mult,
                    accum_out=gdot[:, j:j + 1],
                )
            ln_gss = sp.tile([128, NB], F32, name="ln_gss")
            nc.scalar.activation(ln_gss[:], gss[:], AF.Ln, bias=eps_b[:])
            inv_g = sp.tile([128, NB], F32, name="inv_g")
            nc.scalar.activation(inv_g[:], ln_gss[:], AF.Exp, bias=0.0, scale=-0.5)
            t0 = sp.tile([128, NB], F32, name="t0")
            nc.vector.tensor_tensor(out=t0[:], in0=gdot[:], in1=inv_g[:], op=OP.mult)
            t1 = sp.tile([128, NB], F32, name="t1")
            nc.vector.tensor_tensor(out=t1[:], in0=t0[:], in1=inv_f[:], op=OP.mult)
            t_m = pp.tile([128, NB], F32, name="t_m")
            nc.vector.tensor_tensor(out=t_m[:], in0=t1[:], in1=tmask[:], op=OP.mult)
            tin = dp.tile([128, NB], F32, name="tin")
            tout = dp.tile([128, NB], F32, name="tout", addr_space="Shared")
            nc.sync.dma_start(out=tin[:], in_=t_m[:])
            nc.gpsimd.collective_compute(
                "AllReduce", OP.add,
                replica_groups=[list(range(NCORES))],
                ins=[tin[:]], outs=[tout[:]],
            )
            t_all = pp.tile([128, NB], F32, name="t_all")
            nc.sync.dma_start(out=t_all[:], in_=tout[:])

            # ---- C. weight shard pipeline (bf16): norms + normalize ----
            wTn8 = []
            for P in range(NK // 2):
                w8 = pp.tile([128, 2, CS], FP8, name=f"wTn8{P}")
                wTn8.append(w8)
            for pr in range(NCH // 2):
                c0 = pr * 2 * CHUNK
                wsp2, wsb2, sq2 = [], [], []
                for k in range(NK):
                    wsk = wsp.tile([128, 2 * CHUNK], F32, name="wsk",
                                   tag="wstage")
                    nc.sync.dma_start(
                        out=wsk[:],
                        in_=wt[k * 128:(k + 1) * 128, c0:c0 + 2 * CHUNK])
                    wsp2.append(wsk)
                for k in range(NK):
                    wsbk = sqp.tile([128, 2 * CHUNK], BF16, name="wsbk",
                                    tag="wsb")
                    nc.vector.tensor_copy(out=wsbk[:], in_=wsp2[k][:])
                    wsb2.append(wsbk)
                for k in range(NK):
                    sqk = sqp.tile([128, 2 * CHUNK], BF16, name="sqk", tag="sq")
                    nc.vector.tensor_tensor(out=sqk[:], in0=wsb2[k][:],
                                            in1=wsb2[k][:], op=OP.mult)
                    sq2.append(sqk)
                for half in range(2):
                    cs0 = c0 + half * CHUNK
                    hs = slice(half * CHUNK, (half + 1) * CHUNK)
                    # norm matmul with all-ones lhsT broadcasts the column
                    # sums to every output partition
                    npsum = pn.tile([128, CHUNK], F32, name="npsum",
                                    tag="npsum")
                    for k in range(NK):
                        nc.tensor.matmul(
                            out=npsum[:],
                            lhsT=ones_m[:],
                            rhs=sq2[k][:, hs],
                            start=(k == 0), stop=(k == NK - 1),
                        )
                    lnn = nip.tile([128, CHUNK], F32, name="lnn", tag="lnn")
                    nc.scalar.activation(lnn[:], npsum[:], AF.Ln, bias=eps_b[:])
                    ninvb = nip.tile([128, CHUNK], BF16, name="ninvb",
                                     tag="ninvb")
                    nc.scalar.activation(ninvb[:], lnn[:], AF.Exp,
                                         bias=0.0, scale=-0.5)
                    for k in range(NK):
                        nc.vector.tensor_tensor(
                            out=wTn8[k // 2][:, k % 2, cs0:cs0 + CHUNK],
                            in0=wsb2[k][:, hs], in1=ninvb[:], op=OP.mult)

            # ---- D. main matmuls + fused exp/row-sum epilogue ----
            r_parts = pp.tile([128, NB * NG], F32, name="r_parts")
            r_sum = pp.tile([128, NB], F32, name="r_sum")
            agg_r = pp.tile([128, NB], F32, name="agg_r")
            rins, routs = [], []
            for j in range(NB):
                rins.append(dp.tile([128, 1], F32, name=f"rin{j}"))
                routs.append(dp.tile([128, 1], F32, name=f"rout{j}",
                                     addr_space="Shared"))
            for j in range(NB):
                for q in range(NG // 2):
                    pa = pmm.tile([128, GW, CHUNK], F32, name="pa",
                                  tag="mm_ps", padded_shape=[128, GW, 512])
                    pb = pmm.tile([128, GW, CHUNK], F32, name="pb",
                                  tag="mm_ps", padded_shape=[128, GW, 512])
                    for P in range(NK // 2):
                        lhs = fT8[P][:, :, j * 128:(j + 1) * 128]
                        for t, pt in enumerate((pa, pb)):
                            for cc2 in range(GW):
                                c0 = ((q * 2 + t) * GW + cc2) * CHUNK
                                nc.tensor.matmul(
                                    out=pt[:, cc2, :],
                                    lhsT=lhs,
                                    rhs=wTn8[P][:, :, c0:c0 + CHUNK],
                                    start=(P == 0), stop=(P == NK // 2 - 1),
                                    perf_mode=mybir.MatmulPerfMode.DoubleRow,
                                )
                    for t, pt in enumerate((pa, pb)):
                        g = q * 2 + t
                        expo = xp.tile([128, GW, CHUNK], BF16, name="expo",
                                       tag="expo")
                        nc.scalar.activation(
                            expo[:], pt[:], AF.Exp,
                            bias=negmax_b[:], scale=s_scale[:, j:j + 1],
                            accum_out=r_parts[:, j * NG + g:j * NG + g + 1],
                        )
                nc.vector.reduce_sum(
                    out=r_sum[:, j:j + 1],
                    in_=r_parts[:, j * NG:(j + 1) * NG], axis=AX)
                nc.sync.dma_start(out=rins[j][:], in_=r_sum[:, j:j + 1])
                nc.gpsimd.collective_compute(
                    "AllReduce", OP.add,
                    replica_groups=[list(range(NCORES))],
                    ins=[rins[j][:]], outs=[routs[j][:]],
                )
                nc.sync.dma_start(out=agg_r[:, j:j + 1], in_=routs[j][:])

            # ---- G. phi + corrected log-softmax + mean (redundant on all cores) --
            t2 = sp.tile([128, NB], F32, name="t2")
            nc.vector.tensor_tensor(out=t2[:], in0=t_all[:], in1=t_all[:], op=OP.mult)
            t2c = sp.tile([128, NB], F32, name="t2c")
            nc.vector.tensor_scalar(out=t2c[:], in0=t2[:], scalar1=1.0,
                                    scalar2=None, op0=OP.min)
            ln_u = sp.tile([128, NB], F32, name="ln_u")
            nc.scalar.activation(ln_u[:], t2c[:], AF.Ln, bias=1.0, scale=-1.0)
            sine = sp.tile([128, NB], F32, name="sine")
            nc.scalar.activation(sine[:], ln_u[:], AF.Exp, bias=0.0, scale=0.5)
            pa_ = sp.tile([128, NB], F32, name="pa_")
            nc.vector.tensor_scalar(out=pa_[:], in0=t_all[:], scalar1=COS_M,
                                    scalar2=None, op0=OP.mult)
            pb_ = sp.tile([128, NB], F32, name="pb_")
            nc.vector.tensor_scalar(out=pb_[:], in0=sine[:], scalar1=SIN_M,
                                    scalar2=None, op0=OP.mult)
            phi_m = sp.tile([128, NB], F32, name="phi_m")
            nc.vector.tensor_tensor(out=phi_m[:], in0=pa_[:], in1=pb_[:],
                                    op=OP.subtract)
            phi_alt = sp.tile([128, NB], F32, name="phi_alt")
            nc.vector.tensor_scalar(out=phi_alt[:], in0=t_all[:], scalar1=MM,
                                    scalar2=None, op0=OP.subtract)
            thmask = sp.tile([128, NB], I32, name="thmask")
            nc.vector.tensor_scalar(out=thmask[:], in0=t_all[:], scalar1=TH,
                                    scalar2=None, op0=OP.is_gt)
            phi = sp.tile([128, NB], F32, name="phi")
            nc.vector.select(out=phi[:], mask=thmask[:], on_true=phi_m[:],
                             on_false=phi_alt[:])
            e_t = sp.tile([128, NB], F32, name="e_t")
            nc.scalar.activation(e_t[:], t_all[:], AF.Exp, bias=negmax_b[:], scale=S)
            e_phi = sp.tile([128, NB], F32, name="e_phi")
            nc.scalar.activation(e_phi[:], phi[:], AF.Exp, bias=negmax_b[:], scale=S)
            rc = sp.tile([128, NB], F32, name="rc")
            nc.vector.tensor_tensor(out=rc[:], in0=agg_r[:], in1=e_t[:],
                                    op=OP.subtract)
            rc2 = sp.tile([128, NB], F32, name="rc2")
            nc.vector.tensor_tensor(out=rc2[:], in0=rc[:], in1=e_phi[:], op=OP.add)
            rc3 = sp.tile([128, NB], F32, name="rc3")
            nc.vector.tensor_scalar(out=rc3[:], in0=rc2[:], scalar1=PAD_CORR,
                                    scalar2=None, op0=OP.subtract)
            ln_r = sp.tile([128, NB], F32, name="ln_r")
            nc.scalar.activation(ln_r[:], rc3[:], AF.Ln)
            lp = sp.tile([128, NB], F32, name="lp")
            nc.vector.tensor_scalar(out=lp[:], in0=phi[:], scalar1=S,
                                    scalar2=-MAXL, op0=OP.mult, op1=OP.add)
            ll = sp.tile([128, NB], F32, name="ll")
            nc.vector.tensor_tensor(out=ll[:], in0=ln_r[:], in1=lp[:],
                                    op=OP.subtract)
            lsum = sp.tile([128, 1], F32, name="lsum")
            nc.vector.reduce_sum(out=lsum[:], in_=ll[:], axis=AX)
            loss_ps = pn.tile([1, 1], F32, name="loss_ps", tag="npsum")
            nc.tensor.matmul(out=loss_ps[:], lhsT=lsum[:], rhs=ones_k[:],
                             start=True, stop=True)
            loss_sb = sp.tile([1, 1], F32, name="loss_sb")
            nc.scalar.activation(loss_sb[:], loss_ps[:], AF.Copy, scale=1.0 / B)
            nc.sync.dma_start(out=out_ext[:], in_=loss_sb[:])

    if split_waits:
        _dedup_ldweights(nc)
        _split_excess_waits(nc)
    return nc


_CACHE = {}


def make_in_maps(features, weight, targets):
    feats = np.ascontiguousarray(np.asarray(features, dtype=np.float32))
    W = np.asarray(weight, dtype=np.float32)
    tg = np.asarray(targets).astype(np.int64)

    featT = np.ascontiguousarray(feats.T)
    Wpad = np.zeros((CPAD, D), dtype=np.float32)
    Wpad[:C] = W

    in_maps = []
    for r in range(NCORES):
        wsh = np.ascontiguousarray(Wpad[r * CS:(r + 1) * CS])
        wtr = np.ascontiguousarray(wsh.T)
        tl = (tg - r * CS).astype(np.int32)
        in_maps.append({
            "feat": feats,
            "featT": featT,
            "wt": wtr,
            "wsh": wsh,
            "tloc": tl,
        })
    return in_maps


def kernel(features, weight, targets):
    in_maps = make_in_maps(features, weight, targets)
    if "nc" not in _CACHE:
        _CACHE["nc"] = build_graph()
    nc = _CACHE["nc"]
    res = run_bass_kernel_spmd(nc, in_maps, core_ids=list(range(NCORES)))
    return np.float32(res.results[0]["out"][0, 0])
